# Initial kernel scaffold
#
"""GCN layer (PyG GCNConv + relu + log_softmax) on 8 Trainium2 NeuronCores.

Strategy (per sharding hint):
  - Nodes sharded across 8 cores (12500 each). Core k computes h = x_k @ W,
    g = rsqrt(deg) * h for its shard, AllGathers g so every core holds the
    full normalized feature table in local DRAM.
  - Edges partitioned by destination (host-side bucketing). Within a core,
    destinations are sorted by in-degree (descending) and aggregation runs as
    ELL-style "slot" passes: pass j gathers the j-th source of every active
    destination with an accumulating indirect DMA (unique rows per pass, so
    the read-modify-write accumulate is race-free).
  - Final normalize + bias + relu + log_softmax on-chip; host inverse-permutes
    the per-core results back to node order.
"""

import math
import numpy as np

N_NODES = 100000
N_CORES = 8
SHARD = N_NODES // N_CORES          # 12500
P = 128
C_COLS = math.ceil(SHARD / P)       # 98
SHARD_PAD = C_COLS * P              # 12544
D_IN = 128
D_OUT = 16
GF_ROWS = N_CORES * SHARD_PAD       # 100352 rows in the allgathered table
OOB_IDX = 1 << 22                   # padding index (skipped via bounds_check)

_COMPILED_CACHE = {}


def _build_nc(m_list):
    import concourse.bacc as bacc
    import concourse.bass as bass
    import concourse.mybir as mybir
    from concourse import tile

    M_total = sum(m_list)
    FD = C_COLS * D_OUT  # 1568

    nc = bacc.Bacc(None, target_bir_lowering=False)
    f32 = mybir.dt.float32

    xt_d = nc.dram_tensor("xt", [D_IN, SHARD_PAD], f32, kind="ExternalInput")
    w_d = nc.dram_tensor("w", [D_IN, D_OUT], f32, kind="ExternalInput")
    bia_d = nc.dram_tensor("bia", [P, D_OUT], f32, kind="ExternalInput")
    dega_d = nc.dram_tensor("dega", [P, C_COLS], f32, kind="ExternalInput")
    degb_d = nc.dram_tensor("degb", [P, C_COLS], f32, kind="ExternalInput")
    idx_d = nc.dram_tensor("idx", [P, max(M_total, 1)], mybir.dt.int32,
                           kind="ExternalInput")
    out_d = nc.dram_tensor("out", [SHARD_PAD, D_OUT], f32, kind="ExternalOutput")

    AG = bass.IndirectOffsetOnAxis

    with tile.TileContext(nc) as tc:
        with (
            tc.tile_pool(name="big", bufs=1) as big,
            tc.tile_pool(name="sm", bufs=1) as sm,
            tc.tile_pool(name="work", bufs=2) as work,
            tc.tile_pool(name="ps", bufs=4, space="PSUM") as ps,
            tc.tile_pool(name="dram", bufs=1, space="DRAM") as dram,
        ):
            # ---------------- loads ----------------
            xt_sb = big.tile([D_IN, SHARD_PAD], f32, tag="xt")
            nc.sync.dma_start(out=xt_sb[:], in_=xt_d[:])
            w_sb = sm.tile([D_IN, D_OUT], f32, tag="w")
            nc.sync.dma_start(out=w_sb[:], in_=w_d[:])
            bia_sb = sm.tile([P, D_OUT], f32, tag="bia")
            nc.sync.dma_start(out=bia_sb[:], in_=bia_d[:])
            dega_sb = sm.tile([P, C_COLS], f32, tag="dega")
            nc.sync.dma_start(out=dega_sb[:], in_=dega_d[:])
            degb_sb = sm.tile([P, C_COLS], f32, tag="degb")
            nc.sync.dma_start(out=degb_sb[:], in_=degb_d[:])
            idx_sb = big.tile([P, max(M_total, 1)], mybir.dt.int32, tag="idx")
            nc.sync.dma_start(out=idx_sb[:], in_=idx_d[:])

            disa_sb = sm.tile([P, C_COLS], f32, tag="disa")
            nc.scalar.activation(disa_sb[:], dega_sb[:],
                                 mybir.ActivationFunctionType.Rsqrt)
            disb_sb = sm.tile([P, C_COLS], f32, tag="disb")
            nc.scalar.activation(disb_sb[:], degb_sb[:],
                                 mybir.ActivationFunctionType.Rsqrt)

            # ---------------- phase A: g = dis * (x @ W) ----------------
            g_sb = big.tile([P, C_COLS, D_OUT], f32, tag="g")
            for c in range(C_COLS):
                h_ps = ps.tile([P, D_OUT], f32, tag="h")
                nc.tensor.matmul(h_ps[:], lhsT=xt_sb[:, c * P:(c + 1) * P],
                                 rhs=w_sb[:], start=True, stop=True)
                nc.vector.tensor_scalar_mul(g_sb[:, c, :], h_ps[:],
                                            disa_sb[:, c:c + 1])

            g_shard = dram.tile([SHARD_PAD, D_OUT], f32, tag="gs")
            nc.sync.dma_start(
                out=g_shard[:].rearrange("(c p) f -> p c f", p=P),
                in_=g_sb[:],
            )
            g_full = dram.tile([GF_ROWS, D_OUT], f32, tag="gf")
            nc.gpsimd.collective_compute(
                "AllGather",
                mybir.AluOpType.bypass,
                replica_groups=[list(range(N_CORES))],
                ins=[g_shard[:].opt()],
                outs=[g_full[:].opt()],
            )

            # ---------------- phase B: slot-pass aggregation ----------------
            acc = [work.tile([P, C_COLS, D_OUT], f32, tag=f"acc{i}")
                   for i in range(2)]
            nc.vector.memset(acc[0][:], 0.0)
            nc.vector.memset(acc[1][:], 0.0)
            off = 0
            for j, m in enumerate(m_list):
                a = acc[j % 2]
                nc.gpsimd.indirect_dma_start(
                    out=a[:, :m, :],
                    out_offset=None,
                    in_=g_full[:],
                    in_offset=AG(ap=idx_sb[:, off:off + m], axis=0),
                    bounds_check=GF_ROWS - 1,
                    oob_is_err=False,
                    compute_op=mybir.AluOpType.add,
                )
                off += m

            s_sb = work.tile([P, C_COLS, D_OUT], f32, tag="s")
            nc.vector.tensor_add(s_sb[:], acc[0][:], acc[1][:])

            # out_pre = dis_b * S + b
            disb_b = work.tile([P, C_COLS, D_OUT], f32, tag="bc")
            nc.vector.tensor_copy(
                out=disb_b[:],
                in_=disb_sb[:].rearrange("p c -> p c 1").to_broadcast(
                    [P, C_COLS, D_OUT]),
            )
            nc.vector.tensor_mul(s_sb[:], s_sb[:], disb_b[:])
            bia_b = work.tile([P, C_COLS, D_OUT], f32, tag="bc")
            nc.vector.tensor_copy(
                out=bia_b[:],
                in_=bia_sb[:].rearrange("p f -> p 1 f").to_broadcast(
                    [P, C_COLS, D_OUT]),
            )
            nc.vector.tensor_add(s_sb[:], s_sb[:], bia_b[:])
            nc.scalar.activation(s_sb[:], s_sb[:],
                                 mybir.ActivationFunctionType.Relu)

            # log_softmax over the 16 channels
            mx = sm.tile([P, C_COLS], f32, tag="mx")
            nc.vector.reduce_max(mx[:].rearrange("p c -> p c 1"), s_sb[:],
                                 axis=mybir.AxisListType.X)
            mx_b = work.tile([P, C_COLS, D_OUT], f32, tag="bc")
            nc.vector.tensor_copy(
                out=mx_b[:],
                in_=mx[:].rearrange("p c -> p c 1").to_broadcast(
                    [P, C_COLS, D_OUT]),
            )
            nc.vector.tensor_sub(s_sb[:], s_sb[:], mx_b[:])
            e_sb = work.tile([P, C_COLS, D_OUT], f32, tag="e")
            nc.scalar.activation(e_sb[:], s_sb[:],
                                 mybir.ActivationFunctionType.Exp)
            ssum = sm.tile([P, C_COLS], f32, tag="ssum")
            nc.vector.reduce_sum(ssum[:].rearrange("p c -> p c 1"), e_sb[:],
                                 axis=mybir.AxisListType.X)
            lse = sm.tile([P, C_COLS], f32, tag="lse")
            nc.scalar.activation(lse[:], ssum[:],
                                 mybir.ActivationFunctionType.Ln)
            lse_b = work.tile([P, C_COLS, D_OUT], f32, tag="bc")
            nc.vector.tensor_copy(
                out=lse_b[:],
                in_=lse[:].rearrange("p c -> p c 1").to_broadcast(
                    [P, C_COLS, D_OUT]),
            )
            nc.vector.tensor_sub(s_sb[:], s_sb[:], lse_b[:])

            nc.sync.dma_start(
                out=out_d[:].rearrange("(c p) f -> p c f", p=P),
                in_=s_sb[:],
            )

    nc.finalize()
    return nc


def _prep(x, edge_index, W, b):
    """Host-side sharding/layout prep. Returns (in_maps, perms, m_list)."""
    x = np.asarray(x)
    ei = np.asarray(edge_index)
    W = np.asarray(W, dtype=np.float32)
    b = np.asarray(b, dtype=np.float32)

    row = ei[0].astype(np.int64)   # source
    col = ei[1].astype(np.int64)   # target
    deg = np.bincount(col, minlength=N_NODES).astype(np.int64) + 1

    # g_full row id for a global node n
    gf_row = (np.arange(N_NODES) // SHARD) * SHARD_PAD + (np.arange(N_NODES) % SHARD)

    # per-core edge bucketing by destination shard
    shard_of = col // SHARD
    order = np.argsort(col, kind="stable")
    row_s, col_s = row[order], col[order]

    # per-destination slot lists: slot 0 = self edge, 1.. = in-edges
    # boundaries of each destination in the sorted edge list
    starts = np.searchsorted(col_s, np.arange(N_NODES))
    ends = np.searchsorted(col_s, np.arange(N_NODES) + 1)

    per_core = []
    K_max = 0
    for k in range(N_CORES):
        nodes = np.arange(k * SHARD, (k + 1) * SHARD)
        degs = deg[nodes]  # slots per dest (incl self)
        rank_order = np.argsort(-degs, kind="stable")  # dest nodes by deg desc
        nodes_sorted = nodes[rank_order]
        degs_sorted = degs[rank_order]
        K = int(degs_sorted[0])
        K_max = max(K_max, K)
        per_core.append((nodes_sorted, degs_sorted))

    # pass widths (shared across cores): m_list[j] = max_k ceil(n_active/128)
    m_list = []
    n_active = np.zeros((N_CORES, K_max), dtype=np.int64)
    for k in range(N_CORES):
        _, degs_sorted = per_core[k]
        # n_active[k, j] = number of dests with >= j+1 slots
        cnt = np.bincount(np.minimum(degs_sorted, K_max), minlength=K_max + 1)
        # dests with slots >= j+1  = SHARD - cumsum(cnt[:j+1])
        csum = np.cumsum(cnt)
        for j in range(K_max):
            n_active[k, j] = SHARD - (csum[j] if j < len(csum) else SHARD)
    for j in range(K_max):
        m_list.append(int(math.ceil(max(1, n_active[:, j].max()) / P)))
    M_total = sum(m_list)

    in_maps = []
    perms = []
    for k in range(N_CORES):
        nodes_sorted, degs_sorted = per_core[k]
        # build index array [128, M_total]
        idx = np.full((P, M_total), OOB_IDX, dtype=np.int32)
        # slot sources per dest: self first, then in-edges
        off = 0
        # per dest d (rank r): srcs = [d] + row_s[starts[d]:ends[d]]
        for j, m in enumerate(m_list):
            nact = int(n_active[k, j])
            if nact > 0:
                r = np.arange(nact)
                dests = nodes_sorted[:nact]
                if j == 0:
                    srcs = dests  # self edge
                else:
                    srcs = row_s[starts[dests] + (j - 1)]
                pp = r % P
                cc = r // P
                idx[pp, off + cc] = gf_row[srcs]
            off += m

        # layouts
        xs = x[k * SHARD:(k + 1) * SHARD].astype(np.float32)
        xt = np.zeros((D_IN, SHARD_PAD), dtype=np.float32)
        xt[:, :SHARD] = xs.T
        dega = np.ones((P, C_COLS), dtype=np.float32)
        da = deg[k * SHARD:(k + 1) * SHARD].astype(np.float32)
        dega.T.reshape(-1)[:SHARD] = da  # node l at (p=l%128,c=l//128) -> [c,p]
        dega2 = np.ones((C_COLS, P), dtype=np.float32)
        dega2.reshape(-1)[:SHARD] = da
        degb2 = np.ones((C_COLS, P), dtype=np.float32)
        degb2.reshape(-1)[:SHARD] = deg[nodes_sorted].astype(np.float32)

        in_maps.append({
            "xt": xt,
            "w": W.reshape(D_IN, D_OUT),
            "bia": np.broadcast_to(b, (P, D_OUT)).copy(),
            "dega": dega2.T.copy(),
            "degb": degb2.T.copy(),
            "idx": idx,
        })
        perms.append(nodes_sorted)

    return in_maps, perms, m_list


def kernel(x, edge_index, W, b):
    from concourse.bass_utils import run_bass_kernel_spmd

    in_maps, perms, m_list = _prep(x, edge_index, W, b)

    key = tuple(m_list)
    if key not in _COMPILED_CACHE:
        _COMPILED_CACHE.clear()
        _COMPILED_CACHE[key] = _build_nc(m_list)
    nc = _COMPILED_CACHE[key]

    res = run_bass_kernel_spmd(nc, in_maps, list(range(N_CORES)))

    out = np.zeros((N_NODES, D_OUT), dtype=np.float32)
    for k in range(N_CORES):
        o = res.results[k]["out"]  # [SHARD_PAD, 16] in sorted-rank order
        out[perms[k]] = o[:SHARD]
    return out


# revision 7
# speedup vs baseline: 2.1553x; 2.1553x over previous
"""GCN layer (PyG GCNConv + relu + log_softmax) on 8 Trainium2 NeuronCores.

Strategy (per sharding hint):
  - Nodes sharded across 8 cores (12500 each). Core k computes h = x_k @ W,
    g = rsqrt(deg) * h for its shard, AllGathers g so every core holds the
    full normalized feature table in local DRAM.
  - Edges partitioned by destination (host-side bucketing). Within a core,
    destinations are sorted by in-degree (descending) and aggregation runs as
    ELL-style "slot" passes: pass j gathers the j-th source of every active
    destination with an accumulating indirect DMA (unique rows per pass, so
    the read-modify-write accumulate is race-free).
  - Final normalize + bias + relu + log_softmax on-chip; host inverse-permutes
    the per-core results back to node order.
"""

import math
import numpy as np

N_NODES = 100000
N_CORES = 8
SHARD = N_NODES // N_CORES          # 12500
P = 128
C_COLS = math.ceil(SHARD / P)       # 98
SHARD_PAD = C_COLS * P              # 12544
D_IN = 128
D_OUT = 16
GF_ROWS = N_CORES * SHARD_PAD       # 100352 rows in the allgathered table
OOB_IDX = GF_ROWS                   # padding index -> zeroed pad row of g_full

_COMPILED_CACHE = {}


def _build_nc(m_list):
    import concourse.bacc as bacc
    import concourse.bass as bass
    import concourse.mybir as mybir
    from concourse import tile

    M_total = sum(m_list)
    FD = C_COLS * D_OUT  # 1568

    nc = bacc.Bacc(None, target_bir_lowering=False)
    f32 = mybir.dt.float32

    xt_d = nc.dram_tensor("xt", [D_IN, SHARD_PAD], f32, kind="ExternalInput")
    w_d = nc.dram_tensor("w", [D_IN, D_OUT], f32, kind="ExternalInput")
    bia_d = nc.dram_tensor("bia", [P, D_OUT], f32, kind="ExternalInput")
    dega_d = nc.dram_tensor("dega", [P, C_COLS], f32, kind="ExternalInput")
    degb_d = nc.dram_tensor("degb", [P, C_COLS], f32, kind="ExternalInput")
    idx_d = nc.dram_tensor("idx", [P, max(M_total, 1)], mybir.dt.int32,
                           kind="ExternalInput")
    out_d = nc.dram_tensor("out", [SHARD_PAD, D_OUT], f32, kind="ExternalOutput")

    AG = bass.IndirectOffsetOnAxis

    with tile.TileContext(nc) as tc:
        with (
            tc.tile_pool(name="big", bufs=1) as big,
            tc.tile_pool(name="sm", bufs=1) as sm,
            tc.tile_pool(name="work", bufs=2) as work,
            tc.tile_pool(name="ps", bufs=4, space="PSUM") as ps,
            tc.tile_pool(name="dram", bufs=1, space="DRAM") as dram,
        ):
            # ---------------- loads ----------------
            xt_sb = big.tile([D_IN, SHARD_PAD], f32, tag="xt")
            nc.sync.dma_start(out=xt_sb[:], in_=xt_d[:])
            w_sb = sm.tile([D_IN, D_OUT], f32, tag="w")
            nc.sync.dma_start(out=w_sb[:], in_=w_d[:])
            bia_sb = sm.tile([P, D_OUT], f32, tag="bia")
            nc.sync.dma_start(out=bia_sb[:], in_=bia_d[:])
            dega_sb = sm.tile([P, C_COLS], f32, tag="dega")
            nc.sync.dma_start(out=dega_sb[:], in_=dega_d[:])
            degb_sb = sm.tile([P, C_COLS], f32, tag="degb")
            nc.sync.dma_start(out=degb_sb[:], in_=degb_d[:])
            idx_sb = big.tile([P, max(M_total, 1)], mybir.dt.int32, tag="idx")
            nc.sync.dma_start(out=idx_sb[:], in_=idx_d[:])

            disa_sb = sm.tile([P, C_COLS], f32, tag="disa")
            nc.scalar.activation(disa_sb[:], dega_sb[:],
                                 mybir.ActivationFunctionType.Sqrt)
            nc.vector.reciprocal(disa_sb[:], disa_sb[:])
            disb_sb = sm.tile([P, C_COLS], f32, tag="disb")
            nc.scalar.activation(disb_sb[:], degb_sb[:],
                                 mybir.ActivationFunctionType.Sqrt)
            nc.vector.reciprocal(disb_sb[:], disb_sb[:])

            # ---------------- phase A: g = dis * (x @ W) ----------------
            g_sb = big.tile([P, C_COLS, D_OUT], f32, tag="g")
            for c in range(C_COLS):
                h_ps = ps.tile([P, D_OUT], f32, tag="h")
                nc.tensor.matmul(h_ps[:], lhsT=xt_sb[:, c * P:(c + 1) * P],
                                 rhs=w_sb[:], start=True, stop=True)
                nc.vector.tensor_scalar_mul(g_sb[:, c, :], h_ps[:],
                                            disa_sb[:, c:c + 1])

            g_shard = dram.tile([SHARD_PAD, D_OUT], f32, tag="gs")
            nc.sync.dma_start(
                out=g_shard[:].rearrange("(c p) f -> p c f", p=P),
                in_=g_sb[:],
            )
            g_full = dram.tile([GF_ROWS + P, D_OUT], f32, tag="gf")
            zro = sm.tile([P, D_OUT], f32, tag="zro")
            nc.vector.memset(zro[:], 0.0)
            nc.sync.dma_start(out=g_full[GF_ROWS:, :], in_=zro[:])
            nc.gpsimd.collective_compute(
                "AllGather",
                mybir.AluOpType.bypass,
                replica_groups=[list(range(N_CORES))],
                ins=[g_shard[:].opt()],
                outs=[g_full[:GF_ROWS, :].opt()],
            )

            # ---------------- phase B: slot-pass aggregation ----------------
            acc = [work.tile([P, C_COLS * D_OUT], f32, tag=f"acc{i}",
                             name=f"acc{i}") for i in range(2)]
            nc.vector.memset(acc[0][:], 0.0)
            nc.vector.memset(acc[1][:], 0.0)
            off = 0
            with tc.tile_pool(name="gbuf", bufs=4) as gbuf:
                for j, m in enumerate(m_list):
                    a = acc[j % 2]
                    buf = gbuf.tile([P, m * D_OUT], f32, tag="gb",
                                    name=f"gb{j}")
                    nc.gpsimd.indirect_dma_start(
                        out=buf[:],
                        out_offset=None,
                        in_=g_full[:],
                        in_offset=AG(ap=idx_sb[:, off:off + m], axis=0),
                    )
                    nc.vector.tensor_add(a[:, :m * D_OUT], a[:, :m * D_OUT],
                                         buf[:])
                    off += m

            s_sb = work.tile([P, C_COLS, D_OUT], f32, tag="s")
            nc.vector.tensor_add(
                s_sb[:],
                acc[0][:].rearrange("p (c f) -> p c f", f=D_OUT),
                acc[1][:].rearrange("p (c f) -> p c f", f=D_OUT))

            # out_pre = dis_b * S + b
            disb_b = work.tile([P, C_COLS, D_OUT], f32, tag="bc")
            nc.vector.tensor_copy(
                out=disb_b[:],
                in_=disb_sb[:].broadcast_to([P, C_COLS, D_OUT]),
            )
            nc.vector.tensor_mul(s_sb[:], s_sb[:], disb_b[:])
            bia_b = work.tile([P, C_COLS, D_OUT], f32, tag="bc")
            nc.vector.tensor_copy(
                out=bia_b[:],
                in_=bia_sb[:].rearrange("p (o f) -> p o f", o=1).to_broadcast(
                    [P, C_COLS, D_OUT]),
            )
            nc.vector.tensor_add(s_sb[:], s_sb[:], bia_b[:])
            nc.scalar.activation(s_sb[:], s_sb[:],
                                 mybir.ActivationFunctionType.Relu)

            # log_softmax over the 16 channels
            mx = sm.tile([P, C_COLS], f32, tag="mx")
            nc.vector.reduce_max(mx[:].rearrange("p (c o) -> p c o", o=1), s_sb[:],
                                 axis=mybir.AxisListType.X)
            mx_b = work.tile([P, C_COLS, D_OUT], f32, tag="bc")
            nc.vector.tensor_copy(
                out=mx_b[:],
                in_=mx[:].rearrange("p (c o) -> p c o", o=1).to_broadcast(
                    [P, C_COLS, D_OUT]),
            )
            nc.vector.tensor_sub(s_sb[:], s_sb[:], mx_b[:])
            e_sb = work.tile([P, C_COLS, D_OUT], f32, tag="e")
            nc.scalar.activation(e_sb[:], s_sb[:],
                                 mybir.ActivationFunctionType.Exp)
            ssum = sm.tile([P, C_COLS], f32, tag="ssum")
            nc.vector.reduce_sum(ssum[:].rearrange("p (c o) -> p c o", o=1), e_sb[:],
                                 axis=mybir.AxisListType.X)
            lse = sm.tile([P, C_COLS], f32, tag="lse")
            nc.scalar.activation(lse[:], ssum[:],
                                 mybir.ActivationFunctionType.Ln)
            lse_b = work.tile([P, C_COLS, D_OUT], f32, tag="bc")
            nc.vector.tensor_copy(
                out=lse_b[:],
                in_=lse[:].broadcast_to([P, C_COLS, D_OUT]),
            )
            nc.vector.tensor_sub(s_sb[:], s_sb[:], lse_b[:])

            nc.sync.dma_start(
                out=out_d[:].rearrange("(c p) f -> p c f", p=P),
                in_=s_sb[:],
            )

    nc.finalize()
    return nc


def _prep(x, edge_index, W, b):
    """Host-side sharding/layout prep. Returns (in_maps, perms, m_list)."""
    x = np.asarray(x)
    ei = np.asarray(edge_index)
    W = np.asarray(W, dtype=np.float32)
    b = np.asarray(b, dtype=np.float32)

    row = ei[0].astype(np.int64)   # source
    col = ei[1].astype(np.int64)   # target
    deg = np.bincount(col, minlength=N_NODES).astype(np.int64) + 1

    # g_full row id for a global node n
    gf_row = (np.arange(N_NODES) // SHARD) * SHARD_PAD + (np.arange(N_NODES) % SHARD)

    # per-core edge bucketing by destination shard
    shard_of = col // SHARD
    order = np.argsort(col, kind="stable")
    row_s, col_s = row[order], col[order]

    # per-destination slot lists: slot 0 = self edge, 1.. = in-edges
    # boundaries of each destination in the sorted edge list
    starts = np.searchsorted(col_s, np.arange(N_NODES))
    ends = np.searchsorted(col_s, np.arange(N_NODES) + 1)

    per_core = []
    K_max = 0
    for k in range(N_CORES):
        nodes = np.arange(k * SHARD, (k + 1) * SHARD)
        degs = deg[nodes]  # slots per dest (incl self)
        rank_order = np.argsort(-degs, kind="stable")  # dest nodes by deg desc
        nodes_sorted = nodes[rank_order]
        degs_sorted = degs[rank_order]
        K = int(degs_sorted[0])
        K_max = max(K_max, K)
        per_core.append((nodes_sorted, degs_sorted))

    # pass widths (shared across cores): m_list[j] = max_k ceil(n_active/128)
    m_list = []
    n_active = np.zeros((N_CORES, K_max), dtype=np.int64)
    for k in range(N_CORES):
        _, degs_sorted = per_core[k]
        # n_active[k, j] = number of dests with >= j+1 slots
        cnt = np.bincount(np.minimum(degs_sorted, K_max), minlength=K_max + 1)
        # dests with slots >= j+1  = SHARD - cumsum(cnt[:j+1])
        csum = np.cumsum(cnt)
        for j in range(K_max):
            n_active[k, j] = SHARD - (csum[j] if j < len(csum) else SHARD)
    for j in range(K_max):
        m_list.append(int(math.ceil(max(1, n_active[:, j].max()) / P)))
    M_total = sum(m_list)

    in_maps = []
    perms = []
    for k in range(N_CORES):
        nodes_sorted, degs_sorted = per_core[k]
        # build index array [128, M_total]
        idx = np.full((P, M_total), OOB_IDX, dtype=np.int32)
        # slot sources per dest: self first, then in-edges
        off = 0
        # per dest d (rank r): srcs = [d] + row_s[starts[d]:ends[d]]
        for j, m in enumerate(m_list):
            nact = int(n_active[k, j])
            if nact > 0:
                r = np.arange(nact)
                dests = nodes_sorted[:nact]
                if j == 0:
                    srcs = dests  # self edge
                else:
                    srcs = row_s[starts[dests] + (j - 1)]
                pp = r % P
                cc = r // P
                idx[pp, off + cc] = gf_row[srcs]
            off += m

        # layouts
        xs = x[k * SHARD:(k + 1) * SHARD].astype(np.float32)
        xt = np.zeros((D_IN, SHARD_PAD), dtype=np.float32)
        xt[:, :SHARD] = xs.T
        da = deg[k * SHARD:(k + 1) * SHARD].astype(np.float32)
        dega2 = np.ones((C_COLS, P), dtype=np.float32)
        dega2.reshape(-1)[:SHARD] = da
        degb2 = np.ones((C_COLS, P), dtype=np.float32)
        degb2.reshape(-1)[:SHARD] = deg[nodes_sorted].astype(np.float32)

        in_maps.append({
            "xt": xt,
            "w": W.reshape(D_IN, D_OUT),
            "bia": np.broadcast_to(b, (P, D_OUT)).copy(),
            "dega": dega2.T.copy(),
            "degb": degb2.T.copy(),
            "idx": idx,
        })
        perms.append(nodes_sorted)

    return in_maps, perms, m_list


def kernel(x, edge_index, W, b):
    from concourse.bass_utils import run_bass_kernel_spmd

    in_maps, perms, m_list = _prep(x, edge_index, W, b)

    key = tuple(m_list)
    if key not in _COMPILED_CACHE:
        _COMPILED_CACHE.clear()
        _COMPILED_CACHE[key] = _build_nc(m_list)
    nc = _COMPILED_CACHE[key]

    res = run_bass_kernel_spmd(nc, in_maps, list(range(N_CORES)))

    out = np.zeros((N_NODES, D_OUT), dtype=np.float32)
    for k in range(N_CORES):
        o = res.results[k]["out"]  # [SHARD_PAD, 16] in sorted-rank order
        out[perms[k]] = o[:SHARD]
    return out
